# revision 11
# baseline (speedup 1.0000x reference)
"""Butterfly multiply (n=4096, 12 stages, increasing stride) on 8 Trainium2
NeuronCores.

Math: the 12 butterfly stages factor into two dense matmul passes
  out = C-crossblock @ ( A-blockdiag @ x^T )
where stages 0..6 (strides 1..64) compose into 32 dense 128x128 matrices A_o
acting within 128-aligned blocks, and stages 7..11 (strides 128..2048) compose
into 128 dense 32x32 matrices C_i acting across blocks at fixed within-block
index.  Both are composed on the host from the tiny twiddle input.

v3 strategy (SDMA-pool-aware):
 - x ships per-core as [i=128, o=32, b=1024] fp16: 4 load dma_starts with
   16 KiB-per-partition descriptors (full 16-engine spread, line rate).
 - pass A: stationary A_o^T, moving xT -> psA[i', b] ([128,1024] f32 psum,
   2 matmuls); one contiguous cast copy back over xT per o.
 - permute: per-tau SBUF->SBUF DMA, dst z[:, tau*1024:+1024] (128 dst
   partitions -> all 16 SDMA engines), src xT[4t:4t+4] free-rearranged;
   2 KiB descriptors.  z packing u = a*32+o.
 - pass B: stationary R[tau], moving z-slices -> psB[v = 4*o_out + a, b];
   contiguous cast copy to outb; stores are 4 contiguous dma_starts of
   [128, 8*1024].  Output leaves transposed as ys[v, tau, b]; host unscrambles.
 - PSUM->SBUF copies round-robin over Vector/GpSimd/Scalar so no single
   engine queue serializes; DMA triggers ride sync+scalar HWDGE rings.
 - a dozen dummy matmuls run during the initial load to warm the PE HAM
   clock gate from 1.2 GHz to 2.4 GHz before real work arrives.

Sharding: batch 8192 split across 8 cores (data parallel), twiddle-derived
matrices replicated.
"""

import os
import numpy as np

LOG_N = 12
N = 4096
BATCH = 8192
N_CORES = 8
B_CORE = BATCH // N_CORES  # 1024 rows per core

NWARM = int(os.environ.get("BUTTERFLY_NWARM", "12"))
OGROUP = int(os.environ.get("BUTTERFLY_OGROUP", "8"))  # o's per load/store DMA
COPY_ENGS = os.environ.get("BUTTERFLY_COPY_ENGS", "vs")  # subset of v,g,s


def _compose_matrices(twiddle):
    """Compose stages 0..6 -> A (32,128,128) and stages 7..11 -> C (128,32,32),
    in float64."""
    tw = np.asarray(twiddle)[0, 0].astype(np.float64)  # (12, 2048, 2, 2)

    A = np.zeros((32, 128, 128))
    A[:, np.arange(128), np.arange(128)] = 1.0
    for idx in range(7):
        s = 1 << idx
        Ar = A.reshape(32, 128 // (2 * s), 2, s, 128)  # (o, dl, k, j, i_in)
        o = np.arange(32)[:, None, None]
        dl = np.arange(128 // (2 * s))[None, :, None]
        j = np.arange(s)[None, None, :]
        m = (o * (64 // s) + dl) * s + j
        t = tw[idx, m]  # (32, dl, j, 2, 2)
        x0, x1 = Ar[:, :, 0], Ar[:, :, 1]
        new0 = t[..., 0, 0:1] * x0 + t[..., 0, 1:2] * x1
        new1 = t[..., 1, 0:1] * x0 + t[..., 1, 1:2] * x1
        A = np.stack([new0, new1], axis=2).reshape(32, 128, 128)

    C = np.zeros((128, 32, 32))
    C[:, np.arange(32), np.arange(32)] = 1.0
    for idx in range(7, 12):
        s = 1 << idx
        sp = s // 128
        Cr = C.reshape(128, 32 // (2 * sp), 2, sp, 32)  # (i, dl, k, ol, o_in)
        i = np.arange(128)[None, None, :]
        dl = np.arange(32 // (2 * sp))[:, None, None]
        ol = np.arange(sp)[None, :, None]
        m = dl * (128 * sp) + 128 * ol + i  # (dl, ol, i)
        t = np.moveaxis(tw[idx, m], 2, 0)  # (i, dl, ol, 2, 2)
        x0, x1 = Cr[:, :, 0], Cr[:, :, 1]
        new0 = t[..., 0, 0:1] * x0 + t[..., 0, 1:2] * x1
        new1 = t[..., 1, 0:1] * x0 + t[..., 1, 1:2] * x1
        C = np.stack([new0, new1], axis=2).reshape(128, 32, 32)

    return A, C


def _pack_weights(A, C, np_dt):
    """ATd[i, o*128 + r] = A_o[4*(r%32) + r//32, i]  (pass-A lhsT, a-major rows)
    Rd[u = 4*o_in+a, tau*128 + v = 4*o_out+a] = C[4*tau+a][o_out, o_in]"""
    r = np.arange(128)
    iperm = 4 * (r % 32) + (r // 32)  # row r holds output i' = 4*tau+a, r=a*32+tau
    ATd = np.ascontiguousarray(
        A[:, iperm, :].transpose(2, 0, 1).reshape(128, 32 * 128))

    Rp = np.zeros((32, 128, 128))     # (tau, u, v)
    for tau in range(32):
        for a in range(4):
            Rp[tau, a::4, a::4] = C[4 * tau + a].T
    Rd = np.ascontiguousarray(Rp.transpose(1, 0, 2).reshape(128, 32 * 128))
    return ATd.astype(np_dt), Rd.astype(np_dt)


def _build_program():
    """Trace + compile the per-core Bass program. Returns nc."""
    import concourse.bacc as bacc
    import concourse.tile as tile
    import concourse.mybir as mybir
    from contextlib import ExitStack

    f32 = mybir.dt.float32
    dt = mybir.dt.float16

    nc = bacc.Bacc(
        "TRN2",
        target_bir_lowering=False,
        debug=False,
        enable_asserts=False,
        num_devices=1,
    )
    n_grp = 32 // OGROUP
    x_aps = [
        nc.dram_tensor(f"xt{g}", (128, OGROUP, B_CORE), dt, kind="ExternalInput").ap()
        for g in range(n_grp)
    ]
    at_ap = nc.dram_tensor("AT", (128, 32 * 128), dt, kind="ExternalInput").ap()
    r_ap = nc.dram_tensor("R", (128, 32 * 128), dt, kind="ExternalInput").ap()
    y_aps = [
        nc.dram_tensor(f"y{g}", (128, OGROUP, B_CORE), dt, kind="ExternalOutput").ap()
        for g in range(n_grp)
    ]

    BF = B_CORE            # 1024 batch per core, processed whole
    n_g = 32 // OGROUP     # load/store groups

    def do_copy(k, dst, src):
        eng = COPY_ENGS[k % len(COPY_ENGS)]
        if eng == "s":
            nc.scalar.copy(dst, src)
        elif eng == "g":
            nc.gpsimd.tensor_copy(dst, src)
        else:
            nc.vector.tensor_copy(dst, src)

    with tile.TileContext(nc) as tc, ExitStack() as ctx:
        wpool = ctx.enter_context(tc.tile_pool(name="weights", bufs=1))
        xT_pool = ctx.enter_context(tc.tile_pool(name="xT", bufs=1))
        z_pool = ctx.enter_context(tc.tile_pool(name="z", bufs=1))
        out_pool = ctx.enter_context(tc.tile_pool(name="outb", bufs=2))
        ps_pool = ctx.enter_context(tc.tile_pool(name="ps", bufs=4, space="PSUM"))

        ATw = wpool.tile([128, 32 * 128], dt, tag="ATw")
        Rw = wpool.tile([128, 32 * 128], dt, tag="Rw")
        xT = xT_pool.tile([128, 32 * BF], dt, tag="xT")   # free = o*BF + b
        z = z_pool.tile([128, 32 * BF], dt, tag="z")  # part 4o+a, free t*BF+b

        # loads: ATd first (unblocks warmup+pass A), Rd after x (only needed
        # for pass B).  x in 4 groups with 16 KiB per-partition descriptors.
        nc.sync.dma_start(ATw[:], at_ap)
        for g in range(n_g):
            eng = nc.sync if (g % 2 == 0) else nc.scalar
            eng.dma_start(
                xT[:, g * OGROUP * BF:(g + 1) * OGROUP * BF],
                x_aps[g][:],
            )
        nc.scalar.dma_start(Rw[:], r_ap)

        # PE warmup: keeps the HAM clock gate open while x loads stream in.
        if NWARM > 0:
            warm = ps_pool.tile([128, BF], f32, tag="ps")
            for _ in range(NWARM):
                nc.tensor.matmul(
                    warm[:, 0:512], ATw[:, 0:128], ATw[:, 0:512],
                    start=True, stop=True,
                )
            for _ in range(2 * NWARM):
                nc.tensor.matmul(
                    warm[:, 0:128], ATw[:, 0:128], ATw[:, 0:128],
                    start=True, stop=True,
                )

        # pass A, per o: 2 matmuls into one [128,1024] psum tile + 1 copy
        for o in range(32):
            psA = ps_pool.tile([128, BF], f32, tag="ps")
            for h in range(2):
                nc.tensor.matmul(
                    psA[:, h * 512:(h + 1) * 512],
                    ATw[:, o * 128:(o + 1) * 128],
                    xT[:, o * BF + h * 512:o * BF + (h + 1) * 512],
                    start=True,
                    stop=True,
                )
            do_copy(o, xT[:, o * BF:(o + 1) * BF], psA[:])
            # permute this o's slab immediately (overlaps the rest of pass A).
            # SWDGE takes 2 of 3 (non-blocking ring, drains overlap); the two
            # HWDGE rings take the rest in parallel streams.
            if o % 3 == 2:
                peng = nc.sync if ((o // 3) % 2 == 0) else nc.scalar
            else:
                peng = nc.gpsimd
            peng.dma_start(
                z[4 * o:4 * (o + 1), :].rearrange("a (t b) -> a t b", b=BF),
                xT[:, o * BF:(o + 1) * BF],
            )

        # pass B + store, per tau
        outb = None
        for t in range(32):
            gi = t // OGROUP
            if t % OGROUP == 0:
                outb = out_pool.tile([128, OGROUP * BF], dt, tag="outb")
            psB = ps_pool.tile([128, BF], f32, tag="ps")
            for h in range(2):
                nc.tensor.matmul(
                    psB[:, h * 512:(h + 1) * 512],
                    Rw[:, t * 128:(t + 1) * 128],
                    z[:, t * BF + h * 512:t * BF + (h + 1) * 512],
                    start=True,
                    stop=True,
                )
            do_copy(t + 1, outb[:, (t % OGROUP) * BF:(t % OGROUP + 1) * BF],
                    psB[:])
            if t % OGROUP == OGROUP - 1:
                eng = nc.sync if (gi % 2 == 0) else nc.scalar
                eng.dma_start(y_aps[gi][:], outb[:])

    nc.compile()
    return nc


_CACHE = {}


def _get_program():
    if "nc" not in _CACHE:
        _CACHE["nc"] = _build_program()
    return _CACHE["nc"]


def run(x, twiddle, trace=False, trace_kwargs=None):
    """Run the butterfly kernel on 8 cores. Returns (out, BassKernelResults)."""
    from concourse.bass_utils import run_bass_kernel_spmd

    np_dt = np.float16
    nc = _get_program()

    A, C = _compose_matrices(twiddle)
    ATd, Rd = _pack_weights(A, C, np_dt)

    x = np.asarray(x)
    in_dtype = x.dtype
    xd = x.astype(np_dt)

    n_grp = 32 // OGROUP
    in_maps = []
    for c in range(N_CORES):
        shard = xd[c * B_CORE:(c + 1) * B_CORE]  # (1024, 4096)
        # -> [i, o, b]: shard[b, o*128+i]
        xtc = shard.reshape(B_CORE, 32, 128).transpose(2, 1, 0)
        m = {"AT": ATd, "R": Rd}
        for g in range(n_grp):
            m[f"xt{g}"] = np.ascontiguousarray(
                xtc[:, g * OGROUP:(g + 1) * OGROUP, :]
            )
        in_maps.append(m)

    res = run_bass_kernel_spmd(
        nc,
        in_maps,
        core_ids=list(range(N_CORES)),
        trace=trace,
        **(trace_kwargs or {}),
    )
    # ys[v, tau, b] -> y[b, (v//4)*128 + 4*tau + v%4]
    out = np.empty((BATCH, N), dtype=in_dtype)
    for c in range(N_CORES):
        ys = np.concatenate(
            [np.asarray(res.results[c][f"y{g}"]) for g in range(n_grp)], axis=1
        )  # (128, 32, 1024)
        yb = ys.reshape(32, 4, 32, B_CORE).transpose(3, 0, 2, 1)  # b, o, t, a
        out[c * B_CORE:(c + 1) * B_CORE] = yb.reshape(B_CORE, N)
    return out, res


def kernel(x, twiddle):
    out, _ = run(x, twiddle)
    return out
